# revision 1
# baseline (speedup 1.0000x reference)
"""Tensor-parallel GPT-J-style attention block on 8 TRN2 NeuronCores.

Sharding (vLLM-style TP over heads, plus an all-to-all before the output
projection so the final matmul is token-parallel with no reduction):
  - w_qkv column-sharded: core c computes q/k/v for heads {2c, 2c+1} over all
    tokens (QKV done in transposed layout qkvT = [cols, tok] so the moving
    operand is hidT, which is produced once cooperatively: each core
    transposes its token slice of hidden_states and the slices are
    AllGathered).
  - attention per (batch, head) entirely on-core in [k, q]-transposed layout
    (softmax denominators via a ones-row matmul; causal handled by skipping
    lower blocks and masking the diagonal block).
  - AllToAll reshards attn output from head-sharded to token-sharded; the
    output projection is then a plain matmul against w_out (row-replicated
    via AllGather) with no cross-core reduction.
All matmuls run in float32r (full PE rate at moving-dim >= 256, ~1e-4
relative error vs fp32).
"""
import math
import sys

import numpy as np

try:
    import concourse.bass  # noqa: F401
except ImportError:
    sys.path.insert(0, "/opt/trn_rl_repo")

import concourse.mybir as mybir
import concourse.tile as tile
from concourse import bacc
from concourse.bass_utils import run_bass_kernel_spmd
from concourse.masks import make_identity, make_upper_triangular

dt = mybir.dt

N_CORES = 8
B = 4
NH = 16
D = 256
HID = NH * D  # 4096
ROT = D // 2  # 128
RH = ROT // 2  # 64
HPC = NH // N_CORES  # heads per core
QKV_COLS = 3 * HPC * D  # 1536
SCALE = 1.0 / math.sqrt(D)
ROPE_BASE = 10000.0

_BUILD_CACHE = {}


def build(S, phases=('qkv', 'attn', 'proj'), reps=1):
    TOK = B * S
    TS = TOK // N_CORES  # per-core token slice == QKV token-block width
    assert TS <= 512 and S % TS == 0
    NTB = N_CORES
    KT = HID // 128  # 32 contraction tiles
    NCH = 8  # hidT AllGather chunks
    CH = HID // NCH
    NKT8 = S // 128  # k-token tiles per attention instance
    NQH = max(1, S // 512)  # q halves per attention instance
    QW = min(S, 512)
    f32, f32r = dt.float32, dt.float32r
    RG = [list(range(N_CORES))]

    nc = bacc.Bacc("TRN2", target_bir_lowering=False, debug=False,
                   num_devices=N_CORES)

    # ---- I/O
    pos_in = nc.dram_tensor("pos_f32", [1, TOK], f32, kind="ExternalInput")
    invf_in = nc.dram_tensor("invf", [RH, 1], f32, kind="ExternalInput")
    hid_in = nc.dram_tensor("hid_slice", [TS, HID], f32, kind="ExternalInput")
    wqkv_in = nc.dram_tensor("w_qkv_sh", [HID, QKV_COLS], f32r, kind="ExternalInput")
    wout_in = nc.dram_tensor("w_out_sh", [HID // N_CORES, HID], f32r, kind="ExternalInput")
    out_f = nc.dram_tensor("out_f", [TS, HID], f32, kind="ExternalOutput")

    # ---- internal DRAM
    ag_in = nc.dram_tensor("ag_in", [HID, TS], f32r)
    ag_out = [nc.dram_tensor(f"ag_out{j}", [N_CORES * CH, TS], f32r,
                              addr_space="Shared")
              for j in range(NCH)]
    wag_in = nc.dram_tensor("wag_in", [HID // N_CORES, HID], f32r)
    wout_full = nc.dram_tensor("wout_full", [HID, HID], f32r, addr_space="Shared")
    qkvT_d = nc.dram_tensor("qkvT_d", [2 * HPC * D, TOK], f32r)
    cos_d = nc.dram_tensor("cos_d", [RH, TOK], f32)
    sin_d = nc.dram_tensor("sin_d", [RH, TOK], f32)
    vtok_d = [nc.dram_tensor(f"vtok_d{h}", [TOK, D], f32r) for h in range(HPC)]
    a2a_in = [nc.dram_tensor(f"a2a_in{h}", [N_CORES, D, TS], f32r) for h in range(HPC)]
    a2a_out = [nc.dram_tensor(f"a2a_out{h}", [N_CORES, D, TS], f32r)
               for h in range(HPC)]

    with tile.TileContext(nc) as tc:
        ctx_pool = tc.tile_pool(name="const", bufs=1)
        with ctx_pool as cpool:
            ident = cpool.tile([128, 128], f32)
            make_identity(nc, ident[:])
            ident_r = cpool.tile([128, 128], f32r)
            nc.vector.tensor_copy(out=ident_r[:], in_=ident[:])
            ones_f = cpool.tile([128, 1], f32)
            nc.vector.memset(ones_f[:], 1.0)
            ones_r = cpool.tile([128, 1], f32r)
            nc.vector.tensor_copy(out=ones_r[:], in_=ones_f[:])
            onesrow_f = cpool.tile([1, 128], f32)
            nc.vector.memset(onesrow_f[:], 1.0)
            onesrow_r = cpool.tile([1, 128], f32r)
            nc.vector.tensor_copy(out=onesrow_r[:], in_=onesrow_f[:])
            tri_f = cpool.tile([128, 128], f32)
            make_upper_triangular(nc, tri_f[:], val=1.0, diag=True)
            tri_r = cpool.tile([128, 128], f32r)
            nc.vector.tensor_copy(out=tri_r[:], in_=tri_f[:])
            for rep in range(reps):
                # rope tables ang[i, t] = pos[t] * invf[i], built in TOK
                # chunks and spilled to DRAM (SBUF can't hold [64, TOK] temps).
                two_pi = 2 * math.pi
                TBC = min(TOK, 1024)
                with tc.tile_pool(name=f"rope_tmp_{rep}", bufs=1) as rtp:
                    invf_sb = rtp.tile([RH, 1], f32)
                    nc.sync.dma_start(out=invf_sb[:], in_=invf_in.ap())
                    for chk in range(TOK // TBC):
                        c0 = TBC * chk
                        pos_sb = rtp.tile([1, TBC], f32, tag="pos_sb",
                                          name=f"pos_sb{chk}_r{rep}")
                        nc.sync.dma_start(out=pos_sb[:], in_=pos_in.ap()[:, c0:c0 + TBC])
                        posb = rtp.tile([RH, TBC], f32, tag="posb", name=f"posb{chk}_r{rep}")
                        nc.gpsimd.partition_broadcast(posb[:], pos_sb[0:1, :])
                        ang = rtp.tile([RH, TBC], f32, tag="ang", name=f"ang{chk}_r{rep}")
                        nc.vector.tensor_scalar(out=ang[:], in0=posb[:],
                                                scalar1=invf_sb[:, 0:1], scalar2=None,
                                                op0=mybir.AluOpType.mult)

                        # ScalarE Sin needs [-pi, pi]: f = y - int(y) then fold
                        # f > 0.5 -> f - 1 (works for trunc and round-to-nearest)
                        def reduced_sin(dram_t, shift, nm, chk=chk, c0=c0, ang=ang):
                            y = rtp.tile([RH, TBC], f32, tag="y", name=f"y_{nm}{chk}_r{rep}")
                            nc.vector.tensor_scalar(out=y[:], in0=ang[:],
                                                    scalar1=1.0 / two_pi,
                                                    scalar2=shift / two_pi,
                                                    op0=mybir.AluOpType.mult,
                                                    op1=mybir.AluOpType.add)
                            yi = rtp.tile([RH, TBC], dt.int32, tag="yi",
                                          name=f"yi_{nm}{chk}_r{rep}")
                            nc.vector.tensor_copy(out=yi[:], in_=y[:])
                            yf = rtp.tile([RH, TBC], f32, tag="yf", name=f"yf_{nm}{chk}_r{rep}")
                            nc.vector.tensor_copy(out=yf[:], in_=yi[:])
                            fr = rtp.tile([RH, TBC], f32, tag="fr", name=f"fr_{nm}{chk}_r{rep}")
                            nc.vector.tensor_sub(fr[:], y[:], yf[:])
                            ge = rtp.tile([RH, TBC], f32, tag="ge", name=f"ge_{nm}{chk}_r{rep}")
                            nc.vector.tensor_scalar(out=ge[:], in0=fr[:],
                                                    scalar1=0.5, scalar2=None,
                                                    op0=mybir.AluOpType.is_gt)
                            f2 = rtp.tile([RH, TBC], f32, tag="f2", name=f"f2_{nm}{chk}_r{rep}")
                            nc.vector.tensor_sub(f2[:], fr[:], ge[:])
                            so = rtp.tile([RH, TBC], f32, tag="so", name=f"so_{nm}{chk}_r{rep}")
                            nc.scalar.activation(out=so[:], in_=f2[:],
                                                 func=mybir.ActivationFunctionType.Sin,
                                                 scale=two_pi)
                            nc.sync.dma_start(out=dram_t.ap()[:, c0:c0 + TBC], in_=so[:])

                        reduced_sin(cos_d, math.pi / 2, "c")
                        reduced_sin(sin_d, 0.0, "s")

                # ---- phase 0: transpose own hidden slice, AllGather hidT chunks
                NTT = (TS + 127) // 128
                with tc.tile_pool(name=f"tr_in_{rep}", bufs=1) as tin_pool, \
                     tc.tile_pool(name=f"tr_st_{rep}", bufs=3) as tst_pool, \
                     tc.tile_pool(name=f"tr_ps_{rep}", bufs=2, space="PSUM") as tps_pool:
                    in_tiles = []
                    for tt in range(NTT):
                        pp = min(128, TS - 128 * tt)
                        t = tin_pool.tile([pp, HID], f32, tag=f"tin{tt}")
                        nc.sync.dma_start(out=t[:], in_=hid_in.ap()[128 * tt:128 * tt + pp, :])
                        in_tiles.append((t, pp))
                    for j in range(NCH):
                        for hh in range(CH // 128):
                            h0 = CH * j + 128 * hh
                            for tt in range(NTT):
                                t, pp = in_tiles[tt]
                                pst = tps_pool.tile([128, 128], f32, tag="tp")
                                nc.tensor.transpose(pst[:, 0:pp], t[:, h0:h0 + 128],
                                                    ident[0:pp, 0:pp])
                                st = tst_pool.tile([128, 128], f32r, tag="tst")
                                nc.vector.tensor_copy(out=st[:, 0:pp], in_=pst[:, 0:pp])
                                nc.sync.dma_start(
                                    out=ag_in.ap()[h0:h0 + 128, 128 * tt:128 * tt + pp],
                                    in_=st[:, 0:pp])
                        nc.gpsimd.collective_compute(
                            "AllGather", mybir.AluOpType.bypass, replica_groups=RG,
                            ins=[ag_in.ap()[CH * j:CH * (j + 1), :].opt()],
                            outs=[ag_out[j].ap().opt()])

                # w_out AllGather issued after the hidT chunks (overlaps qkv+attn)
                nc.sync.dma_start(out=wag_in.ap(), in_=wout_in.ap())
                nc.gpsimd.collective_compute(
                    "AllGather", mybir.AluOpType.bypass, replica_groups=RG,
                    ins=[wag_in.ap().opt()], outs=[wout_full.ap().opt()])

                # ---- phase 1: QKV projection (transposed), two column halves
                NM = 6
                qkv_passes = range(2) if 'qkv' in phases else range(0)
                ROPE_GC = {g * 256 for g in range(2 * HPC)}  # first 128 cols of q/k blocks
                for p in qkv_passes:
                    with tc.tile_pool(name=f"qkv_w{p}_{rep}", bufs=1) as wq_pool, \
                         tc.tile_pool(name=f"qkv_rhs{p}_{rep}", bufs=3) as rhs_pool, \
                         tc.tile_pool(name=f"qkv_st{p}_{rep}", bufs=2) as stg_pool, \
                         tc.tile_pool(name=f"qkv_ps{p}_{rep}", bufs=1, space="PSUM") as qps_pool:
                        w_sb = []
                        for kt in range(KT):
                            w = wq_pool.tile([128, NM * 128], f32r, tag=f"w{kt}")
                            nc.sync.dma_start(
                                out=w[:],
                                in_=wqkv_in.ap()[128 * kt:128 * (kt + 1),
                                                 NM * 128 * p:NM * 128 * (p + 1)])
                            w_sb.append(w)
                        KB = 8  # k-tiles per rhs block: same-psum matmul runs
                        KPC = CH // 128  # k-tiles per AllGather chunk
                        for tb in range(NTB):
                            ps = [qps_pool.tile([128, TS], dt.float32, tag=f"qkvps{m}",
                                                name=f"qkvps{m}_{p}_{tb}_r{rep}")
                                  for m in range(NM)]
                            # rotate the contraction order per token-block so
                            # the first pass chases the AllGather wavefront
                            # instead of every block stalling on the last chunk
                            kseq = [(KPC * tb + i) % KT for i in range(KT)]
                            for kb in range(KT // KB):
                                kts_blk = kseq[KB * kb:KB * (kb + 1)]
                                blk = rhs_pool.tile([128, KB * TS], f32r, tag="qkvrhs",
                                                    name=f"qkvrhs_{p}_{tb}_{kb}_r{rep}")
                                for i, kt in enumerate(kts_blk):
                                    j, r8 = kt // (CH // 128), kt % (CH // 128)
                                    row0 = CH * tb + 128 * r8
                                    nc.sync.dma_start(
                                        out=blk[:, TS * i:TS * (i + 1)],
                                        in_=ag_out[j].ap()[row0:row0 + 128, :])
                                for m in range(NM):
                                    for i, kt in enumerate(kts_blk):
                                        nc.tensor.matmul(
                                            out=ps[m][:],
                                            lhsT=w_sb[kt][:, 128 * m:128 * (m + 1)],
                                            rhs=blk[:, TS * i:TS * (i + 1)],
                                            start=(kb == 0 and i == 0),
                                            stop=(kb == KT // KB - 1 and i == KB - 1))
                            for m in range(NM):
                                gc = NM * 128 * p + 128 * m
                                dst = stg_pool.tile([128, TS], f32r, tag=f"qst{m}")
                                if gc in ROPE_GC:
                                    cs_ = stg_pool.tile([RH, TS], f32, tag="cs",
                                                        name=f"cs_{p}_{tb}_{m}_r{rep}")
                                    sn_ = stg_pool.tile([RH, TS], f32, tag="sn",
                                                        name=f"sn_{p}_{tb}_{m}_r{rep}")
                                    nc.sync.dma_start(out=cs_[:],
                                                      in_=cos_d.ap()[:, TS * tb:TS * (tb + 1)])
                                    nc.sync.dma_start(out=sn_[:],
                                                      in_=sin_d.ap()[:, TS * tb:TS * (tb + 1)])
                                    c = cs_[:]
                                    s = sn_[:]
                                    t1 = stg_pool.tile([RH, TS], f32, tag="rt1")
                                    t2 = stg_pool.tile([RH, TS], f32, tag="rt2")
                                    nc.vector.tensor_mul(t1[:], ps[m][0:RH, :], c)
                                    nc.vector.tensor_mul(t2[:], ps[m][RH:2 * RH, :], s)
                                    nc.vector.tensor_sub(dst[0:RH, :], t1[:], t2[:])
                                    t3 = stg_pool.tile([RH, TS], f32, tag="rt3")
                                    t4 = stg_pool.tile([RH, TS], f32, tag="rt4")
                                    nc.vector.tensor_mul(t3[:], ps[m][RH:2 * RH, :], c)
                                    nc.vector.tensor_mul(t4[:], ps[m][0:RH, :], s)
                                    nc.vector.tensor_add(dst[RH:2 * RH, :], t3[:], t4[:])
                                else:
                                    nc.vector.tensor_copy(out=dst[:], in_=ps[m][:])
                                if gc < 2 * HPC * D:
                                    nc.sync.dma_start(
                                        out=qkvT_d.ap()[gc:gc + 128, TS * tb:TS * (tb + 1)],
                                        in_=dst[:])
                                else:
                                    hl_ = (gc - 2 * HPC * D) // D
                                    d0 = (gc - 2 * HPC * D) % D
                                    for q8 in range((TS + 127) // 128):
                                        qq = min(128, TS - 128 * q8)
                                        tpv = qps_pool.tile([128, 128], f32r, tag="vtp",
                                                            name=f"vtp_{p}_{tb}_{m}_{q8}_r{rep}",
                                                            bufs=2)
                                        nc.tensor.transpose(
                                            tpv[0:qq, :],
                                            dst[:, 128 * q8:128 * q8 + qq],
                                            ident_r[:])
                                        vst = stg_pool.tile([128, 128], f32r, tag="vst")
                                        nc.vector.tensor_copy(out=vst[0:qq, :],
                                                              in_=tpv[0:qq, :])
                                        tok0 = TS * tb + 128 * q8
                                        nc.sync.dma_start(
                                            out=vtok_d[hl_].ap()[tok0:tok0 + qq, d0:d0 + 128],
                                            in_=vst[0:qq, :])

                # ---- phase 2: attention per (local head, batch)
                attn_on = 'attn' in phases
                with tc.tile_pool(name=f"att_in_{rep}", bufs=2) as ain_pool, \
                     tc.tile_pool(name=f"att_vt_{rep}", bufs=2) as avt_pool, \
                     tc.tile_pool(name=f"att_pr_{rep}", bufs=2) as apr_pool, \
                     tc.tile_pool(name=f"att_o_{rep}", bufs=2) as aout_pool, \
                     tc.tile_pool(name=f"att_sc_{rep}", bufs=2, space="PSUM") as scps_pool, \
                     tc.tile_pool(name=f"att_av_{rep}", bufs=1, space="PSUM") as avps_pool:
                    for hl in (range(HPC) if attn_on else range(0)):
                        for b in range(B):
                            qoff = D * hl
                            koff = HPC * D + D * hl
                            tok0 = S * b

                            def load_pair(off, nm):
                                ts_ = []
                                for dtile in range(2):
                                    t = ain_pool.tile([128, S], f32r, tag=f"{nm}{dtile}")
                                    nc.sync.dma_start(
                                        out=t[:],
                                        in_=qkvT_d.ap()[off + 128 * dtile:off + 128 * (dtile + 1),
                                                        tok0:tok0 + S])
                                    ts_.append(t)
                                return ts_

                            qT = load_pair(qoff, "q")
                            kT = load_pair(koff, "k")

                            # token-major v tiles (transposed during QKV phase)
                            vtok = []
                            for kt8 in range(NKT8):
                                vt = avt_pool.tile([128, D], f32r, tag=f"vtok{kt8}")
                                nc.sync.dma_start(
                                    out=vt[:],
                                    in_=vtok_d[hl].ap()[tok0 + 128 * kt8:tok0 + 128 * (kt8 + 1), :])
                                vtok.append(vt)

                            # scoresT -> exp -> probsT
                            probsT = []
                            for kt8 in range(NKT8):
                                pr = apr_pool.tile([128, S], f32r, tag=f"pr{kt8}")
                                qlo = 128 * kt8
                                q0 = qlo
                                while q0 < S:
                                    wch = min(512, S - q0)
                                    pss = scps_pool.tile([128, QW], dt.float32, tag="scps")
                                    for dtile in range(2):
                                        nc.tensor.matmul(
                                            out=pss[:, 0:wch],
                                            lhsT=kT[dtile][:, 128 * kt8:128 * (kt8 + 1)],
                                            rhs=qT[dtile][:, q0:q0 + wch],
                                            start=(dtile == 0), stop=(dtile == 1))
                                    nc.scalar.activation(
                                        out=pr[:, q0:q0 + wch], in_=pss[:, 0:wch],
                                        func=mybir.ActivationFunctionType.Exp, scale=SCALE)
                                    q0 += wch
                                nc.vector.tensor_mul(pr[:, qlo:qlo + 128],
                                                     pr[:, qlo:qlo + 128], tri_r[:])
                                probsT.append(pr)

                            # PV + denominator
                            ps_av = [[avps_pool.tile([128, QW], dt.float32, tag=f"av{d}{q}",
                                                      name=f"av{d}{q}_{hl}_{b}_r{rep}")
                                      for q in range(NQH)] for d in range(2)]
                            ps_sum = [avps_pool.tile([1, QW], dt.float32, tag=f"sm{q}",
                                                     name=f"sm{q}_{hl}_{b}_r{rep}")
                                      for q in range(NQH)]
                            pv_work = {}
                            for qh in range(NQH):
                                q0, q1 = QW * qh, QW * (qh + 1)
                                last_kt = min(NKT8 - 1, (q1 - 1) // 128)
                                pv_work[qh] = [
                                    (kt8, q0, q1, max(128 * kt8, q0),
                                     kt8 == 0, kt8 == last_kt)
                                    for kt8 in range(NKT8)
                                    if max(128 * kt8, q0) < q1]
                            for dtile in range(2):
                                for qh in range(NQH):
                                    for kt8, q0, q1, lo, st, sp in pv_work[qh]:
                                        nc.tensor.matmul(
                                            out=ps_av[dtile][qh][:, lo - q0:q1 - q0],
                                            lhsT=vtok[kt8][:, 128 * dtile:128 * (dtile + 1)],
                                            rhs=probsT[kt8][:, lo:q1], start=st, stop=sp)
                            for qh in range(NQH):
                                for kt8, q0, q1, lo, st, sp in pv_work[qh]:
                                    nc.tensor.matmul(out=ps_sum[qh][:, lo - q0:q1 - q0],
                                                     lhsT=ones_r[:],
                                                     rhs=probsT[kt8][:, lo:q1],
                                                     start=st, stop=sp)

                            # normalize and scatter to a2a input
                            sums_sb = aout_pool.tile([1, S], f32, tag="sums")
                            for qh in range(NQH):
                                nc.scalar.copy(out=sums_sb[:, QW * qh:QW * (qh + 1)],
                                               in_=ps_sum[qh][:])
                            recip = aout_pool.tile([1, S], f32, tag="recip")
                            nc.vector.reciprocal(out=recip[:], in_=sums_sb[:])
                            recip_r = aout_pool.tile([1, S], f32r, tag="recip_r",
                                                     name=f"recipr_{hl}_{b}_r{rep}")
                            nc.vector.tensor_copy(out=recip_r[:], in_=recip[:])
                            rbc = aout_pool.tile([128, S], f32, tag="rbc")
                            for qh in range(NQH):
                                bcp = scps_pool.tile([128, QW], dt.float32, tag="scps",
                                                     name=f"bcp_{hl}_{b}_{qh}_r{rep}")
                                nc.tensor.matmul(out=bcp[:],
                                                 lhsT=onesrow_r[:],
                                                 rhs=recip_r[:, QW * qh:QW * (qh + 1)],
                                                 start=True, stop=True)
                                nc.vector.tensor_copy(
                                    out=rbc[:, QW * qh:QW * (qh + 1)], in_=bcp[:])
                            for dtile in range(2):
                                att_sb = aout_pool.tile([128, S], f32r, tag=f"attn{dtile}")
                                for qh in range(NQH):
                                    q0, q1 = QW * qh, QW * (qh + 1)
                                    nc.vector.tensor_mul(att_sb[:, q0:q1],
                                                         ps_av[dtile][qh][:],
                                                         rbc[:, q0:q1])
                                for u in range(S // TS):
                                    dest = (S * b) // TS + u
                                    nc.sync.dma_start(
                                        out=a2a_in[hl].ap()[dest,
                                                            128 * dtile:128 * (dtile + 1), :],
                                        in_=att_sb[:, TS * u:TS * (u + 1)])
                        nc.gpsimd.collective_compute(
                            "AllToAll", mybir.AluOpType.bypass, replica_groups=RG,
                            ins=[a2a_in[hl].ap().opt()], outs=[a2a_out[hl].ap().opt()])

                # ---- phase 3: output projection for own token slice
                NMT = (TS + 127) // 128
                NNT = HID // 512 if 'proj' in phases else 0
                with tc.tile_pool(name=f"op_a_{rep}", bufs=1) as oa_pool, \
                     tc.tile_pool(name=f"op_w_{rep}", bufs=2) as ow_pool, \
                     tc.tile_pool(name=f"op_f_{rep}", bufs=3) as of_pool, \
                     tc.tile_pool(name=f"op_ps_{rep}", bufs=1, space="PSUM") as ops_pool:
                    am = {}
                    for hl in (range(HPC) if 'proj' in phases else range(0)):
                        for src in range(N_CORES):
                            for sub in range(2):
                                t = oa_pool.tile([128, TS], f32r, tag=f"am{hl}_{src}_{sub}")
                                nc.sync.dma_start(
                                    out=t[:],
                                    in_=a2a_out[hl].ap()[src, 128 * sub:128 * (sub + 1), :])
                                am[(hl, src, sub)] = t
                    kts = [(hl, src, sub) for hl in range(HPC)
                           for src in range(N_CORES) for sub in range(2)]
                    PKB = 4  # k-tiles per w block
                    for nt in range(NNT):
                        ps_f = [ops_pool.tile([min(128, TS), 512], dt.float32, tag=f"f{mt}",
                                              name=f"f{mt}_{nt}_r{rep}")
                                for mt in range(NMT)]
                        for kb in range((len(kts) + PKB - 1) // PKB):
                            kis = list(range(PKB * kb, min(PKB * (kb + 1), len(kts))))
                            wblk = ow_pool.tile([128, PKB * 512], f32r, tag="wr",
                                                name=f"wr_{nt}_{kb}_r{rep}")
                            for i, ki in enumerate(kis):
                                hl, src, sub = kts[ki]
                                r0 = 512 * src + 256 * hl + 128 * sub
                                nc.sync.dma_start(
                                    out=wblk[:, 512 * i:512 * (i + 1)],
                                    in_=wout_full.ap()[r0:r0 + 128,
                                                       512 * nt:512 * (nt + 1)])
                            for mt in range(NMT):
                                mm = min(128, TS - 128 * mt)
                                for i, ki in enumerate(kis):
                                    nc.tensor.matmul(
                                        out=ps_f[mt][:],
                                        lhsT=am[kts[ki]][:, 128 * mt:128 * mt + mm],
                                        rhs=wblk[:, 512 * i:512 * (i + 1)],
                                        start=(ki == 0), stop=(ki == len(kts) - 1))
                        for mt in range(NMT):
                            mm = min(128, TS - 128 * mt)
                            fo = of_pool.tile([min(128, TS), 512], f32, tag="fo")
                            nc.scalar.copy(out=fo[:], in_=ps_f[mt][:])
                            nc.sync.dma_start(
                                out=out_f.ap()[128 * mt:128 * mt + mm, 512 * nt:512 * (nt + 1)],
                                in_=fo[:])

    nc.compile()
    return nc


def get_nc(S):
    if S not in _BUILD_CACHE:
        _BUILD_CACHE[S] = build(S)
    return _BUILD_CACHE[S]


def make_in_maps(position_ids, hidden_states, w_qkv, w_out):
    S = hidden_states.shape[1]
    TOK = B * S
    TS = TOK // N_CORES
    flat = np.asarray(hidden_states, dtype=np.float32).reshape(TOK, HID)
    pos = np.asarray(position_ids).reshape(1, TOK).astype(np.float32)
    invf = (1.0 / (ROPE_BASE ** (np.arange(0, ROT, 2, dtype=np.float32) / ROT)))
    invf = invf.reshape(RH, 1).astype(np.float32)
    w_qkv = np.asarray(w_qkv, dtype=np.float32)
    w_out = np.asarray(w_out, dtype=np.float32)
    rows_per = HID // N_CORES
    in_maps = []
    for c in range(N_CORES):
        c0 = HPC * D * c
        wq = np.concatenate([w_qkv[:, c0:c0 + HPC * D],
                             w_qkv[:, HID + c0:HID + c0 + HPC * D],
                             w_qkv[:, 2 * HID + c0:2 * HID + c0 + HPC * D]], axis=1)
        in_maps.append({
            "pos_f32": pos,
            "invf": invf,
            "hid_slice": np.ascontiguousarray(flat[TS * c:TS * (c + 1)]),
            "w_qkv_sh": np.ascontiguousarray(wq),
            "w_out_sh": np.ascontiguousarray(w_out[rows_per * c:rows_per * (c + 1)]),
        })
    return in_maps


def kernel(position_ids, hidden_states, w_qkv, w_out):
    S = hidden_states.shape[1]
    nc = get_nc(S)
    in_maps = make_in_maps(position_ids, hidden_states, w_qkv, w_out)
    res = run_bass_kernel_spmd(nc, in_maps, list(range(N_CORES)))
    TOK = B * S
    out = np.concatenate([res.results[c]["out_f"] for c in range(N_CORES)], axis=0)
    return out.reshape(B, S, HID).astype(np.float32)

